# revision 1
# baseline (speedup 1.0000x reference)
"""Trainium2 Bass kernel for the LTPE block:

    out_j = conv3x3(x, kernel_j)   (8 kernels: [-1 at neighbor j, +1 at center])
    out   = sum_j ((out_j + 1) * 0.5) * (2**j / 255)
    out   = InstanceNorm2d(out)    (per-sample over H,W, eps=1e-5, no affine)

Math: sum_j 2**j/255 == 1, so
    out = 0.5*(x - conv) + 0.5,  conv = sum_j (2**j/255) * shift_j(x)
InstanceNorm is invariant to the affine: with z = 255*x - sum_j 2**j*shift_j(x)
    result = (z - mean(z)) / sqrt(var(z) + 260100e-5)
z is computed as a 3x3 stencil via banded [128,128] fp32 matmuls (one per
column shift; walrus lowers fp32 matmuls to HI/LO passes on the PE, keeping
near-fp32 accuracy with no on-chip operand splitting).  Pure data parallel:
4 samples per NeuronCore, 8 cores.

Row tiling: tile t computes output rows [126t, 126t+126) (last tile: 16 rows)
from input rows [126t-1, 126t+127).  Output row 126t+n sits at partition n;
the vertical taps form a banded matrix with band (0,1,2) for t>0 and
(-1,0,1) for t=0 (zero-pad rows handled by band clipping / K=17 on the tail).

Samples are software-pipelined at tile granularity: the finalize chain of
sample s-1 (stats aggregation, normalize, store) is emitted in small chunks
between the tile emissions of sample s.  Input loads and output stores are
split across both HWDGE queues (sync + scalar engines).
"""

import numpy as np

import concourse.bass as bass
import concourse.tile as tile
from concourse import mybir
from concourse.bacc import Bacc
from concourse.bass_utils import run_bass_kernel_spmd

N_CORES = 8
B_PER_CORE = 4
H = W = 1024
TO = 126           # output rows per tile (input rows = TO + 2 halo)
NT = 9             # 8 full tiles + 16-row tail
TAIL = H - 8 * TO  # 16
EPS_P = 260100e-5  # 255^2 * 4 * 1e-5 : the InstanceNorm eps after rescaling

# neighbor offsets (dy, dx) for weights 2**j
_OFFSETS = [(0, -1), (1, -1), (1, 0), (1, 1), (0, 1), (-1, 1), (-1, 0), (-1, -1)]

F32 = mybir.dt.float32
ALU = mybir.AluOpType
AF = mybir.ActivationFunctionType


def _build_host_weights():
    """Banded matrices V[dx][k, n]: coefficient of input partition k for
    output partition n, for column shift dx.  Band "a" (t=0): input row at
    partition k is row k, out row n -> taps k=n+dy.  Band "b" (t>0): input
    row at partition k is 126t-1+k, out row 126t+n -> taps k=n+1+dy."""
    out = {}
    for name, shift in (("a", 0), ("b", 1)):
        V = {dx: np.zeros((128, 128), np.float32) for dx in (-1, 0, 1)}
        for n in range(128):
            k = n + shift
            if k < 128:
                V[0][k, n] = 255.0  # center tap (+255 x)
        for j, (dy, dx) in enumerate(_OFFSETS):
            for n in range(128):
                k = n + shift + dy
                if 0 <= k < 128:
                    V[dx][k, n] += -float(2 ** j)
        for dx, tag in ((-1, "l"), (0, "c"), (1, "r")):
            out[f"v{tag}{name}"] = np.ascontiguousarray(V[dx], dtype=np.float32)

    # cross-partition count weights: row k weighted n_k / (H*W); all 128
    # output columns identical -> the matmul broadcasts the totals.
    counts = np.zeros((128,), np.float64)
    for t in range(NT):
        n_out = TO if t < 8 else TAIL
        counts[0:n_out] += W
    wcnt = np.tile((counts / float(H * W)).astype(np.float32)[:, None], (1, 128))
    out["wcnt"] = np.ascontiguousarray(wcnt, dtype=np.float32)
    return out


def _mm_cols(vname, h):
    """(in_c0, in_c1, out_c0, out_c1) for weight vname on PSUM half h:
    column shifts realized by sliding the moving operand's columns."""
    c0 = 512 * h
    if vname == "vc":
        return (c0, c0 + 512, 0, 512)
    if vname == "vl":
        return (0, 511, 1, 512) if h == 0 else (511, 1023, 0, 512)
    return (1, 513, 0, 512) if h == 0 else (513, 1024, 0, 511)


def build_nc(mode="fp32", lo_passes=None):
    nc = Bacc()
    x_in = nc.declare_dram_parameter("x", [B_PER_CORE, 1, H, W], F32, isOutput=False)
    out_ext = nc.declare_dram_parameter("out", [B_PER_CORE, 1, H, W], F32, isOutput=True)
    w_names = ["vla", "vca", "vra", "vlb", "vcb", "vrb"]
    w_dram = {
        n: nc.declare_dram_parameter(n, [128, 128], F32, isOutput=False)
        for n in w_names
    }
    wcnt_d = nc.declare_dram_parameter("wcnt", [128, 128], F32, isOutput=False)

    def in_rows(t):
        in_a = max(TO * t - 1, 0)
        in_b = min(TO * t + TO + 1, H)
        return in_a, in_b

    def in_dma_engine(t):
        return nc.sync if t % 2 == 0 else nc.scalar

    with tile.TileContext(nc) as tc:
        with (
            tc.tile_pool(name="singles", bufs=1) as singles,
            tc.tile_pool(name="xp", bufs=10) as xp,
            tc.tile_pool(name="zp", bufs=2) as zp,
            tc.tile_pool(name="stat", bufs=2) as stat,
            tc.tile_pool(name="sm", bufs=4) as sm,
            tc.tile_pool(name="psp", bufs=3, space="PSUM") as psp,
            tc.tile_pool(name="pss", bufs=1, space="PSUM") as pss,
        ):
            # prefetch the first tiles of sample 0 ahead of the weight loads
            prefetched = {}
            for pf_t in (0, 1, 2, 3):
                in_a, in_b = in_rows(pf_t)
                xt = xp.tile([128, W], F32, tag="xt", name=f"xt_pf{pf_t}")
                in_dma_engine(pf_t).dma_start(
                    out=xt[0:in_b - in_a, :], in_=x_in[0, 0, in_a:in_b, :]
                )
                prefetched[(0, pf_t)] = xt

            sb_v = {}
            for n in w_names:
                t_ = singles.tile([128, 128], F32, tag=n)
                nc.sync.dma_start(out=t_, in_=w_dram[n][:, :])
                sb_v[n] = t_
            sb_wcnt = singles.tile([128, 128], F32, tag="wcnt")
            nc.sync.dma_start(out=sb_wcnt, in_=wcnt_d[:, :])
            sb_eps = singles.tile([128, 1], F32, tag="eps")
            nc.vector.memset(sb_eps, EPS_P)

            def emit_tile(s, t, z_big, stats):
                if t == 0:
                    nc.gpsimd.memset(stats[:], 0.0)
                n_out = TO if t < 8 else TAIL
                in_a, in_b = in_rows(t)
                rows = in_b - in_a              # 127/128, or 17 on the tail
                K = rows                        # contraction depth
                band = "a" if t == 0 else "b"

                if (s, t) in prefetched:
                    xt = prefetched.pop((s, t))
                else:
                    xt = xp.tile([128, W], F32, tag="xt")
                    in_dma_engine(t).dma_start(
                        out=xt[0:rows, :], in_=x_in[s, 0, in_a:in_b, :]
                    )

                ps = psp.tile([128, 2, 512], F32, tag="ps")
                plan = [(v, h) for v in ("vc", "vl", "vr") for h in (0, 1)]
                for i, (vname, h) in enumerate(plan):
                    a, b, oa, ob = _mm_cols(vname, h)
                    nc.tensor.matmul(
                        ps[:, h, oa:ob],
                        lhsT=sb_v[vname + band][0:K, :],
                        rhs=xt[0:K, a:b],
                        start=(i < 2),
                        stop=(i >= len(plan) - 2),
                        skip_group_check=True,
                    )

                nc.scalar.copy(
                    out=z_big[0:n_out, t, :].rearrange("p (g f) -> p g f", f=512),
                    in_=ps[0:n_out, :, :],
                )
                for g in (0, 1):
                    nc.vector.bn_stats(
                        out=stats[0:n_out, t, g, :],
                        in_=ps[0:n_out, g, :],
                    )

            def finalize_chunks(s, z_big, stats):
                box = {}

                def c1():
                    mv = box["mv"] = sm.tile([128, 2], F32, tag="mv", name="mv")
                    nc.vector.memset(mv, 0.0)
                    nc.vector.bn_aggr(out=mv[0:TO, :], in_=stats[0:TO, :, :, :])
                    msq = sm.tile([128, 1], F32, tag="msq")
                    nc.vector.tensor_mul(msq, mv[:, 0:1], mv[:, 0:1])
                    nc.vector.tensor_add(mv[:, 1:2], mv[:, 1:2], msq)  # E2

                def c2():
                    tot_ps = pss.tile([128, 2], F32, tag="totps")
                    nc.tensor.matmul(
                        tot_ps[:, :], lhsT=sb_wcnt[:, :], rhs=box["mv"][:, :],
                        start=True, stop=True,
                    )
                    tot = box["tot"] = sm.tile([128, 2], F32, tag="tot", name="tot")
                    nc.scalar.copy(out=tot, in_=tot_ps)

                def c3():
                    tot = box["tot"]
                    m2 = sm.tile([128, 1], F32, tag="m2")
                    nc.vector.tensor_mul(m2, tot[:, 0:1], tot[:, 0:1])
                    var = sm.tile([128, 1], F32, tag="var")
                    nc.vector.tensor_sub(var, tot[:, 1:2], m2)
                    sd = box["sd"] = sm.tile([128, 1], F32, tag="sd", name="sd")
                    nc.scalar.activation(
                        out=sd, in_=var, func=AF.Sqrt, bias=sb_eps, scale=1.0
                    )

                def c4():
                    inv = box["inv"] = sm.tile([128, 1], F32, tag="inv", name="inv")
                    nc.vector.reciprocal(inv, box["sd"])
                    nbias = box["nb"] = sm.tile([128, 1], F32, tag="nb", name="nb")
                    nc.vector.tensor_scalar(
                        out=nbias, in0=inv, scalar1=box["tot"][:, 0:1],
                        scalar2=-1.0, op0=ALU.mult, op1=ALU.mult,
                    )

                def norm_store(t0, t1, eng_name):
                    def c():
                        nc.vector.tensor_scalar(
                            out=z_big[0:TO, t0:t1, :], in0=z_big[0:TO, t0:t1, :],
                            scalar1=box["inv"][0:TO, :],
                            scalar2=box["nb"][0:TO, :],
                            op0=ALU.mult, op1=ALU.add,
                        )
                        # output row 126t+n <-> (n, t) of z_big
                        eng = nc.sync if eng_name == "sync" else nc.scalar
                        eng.dma_start(
                            out=out_ext[s, 0, TO * t0:TO * t1, :].rearrange(
                                "(t n) w -> n t w", n=TO
                            ),
                            in_=z_big[0:TO, t0:t1, :],
                        )
                    return c

                def c_tail():
                    nc.vector.tensor_scalar(
                        out=z_big[0:TAIL, 8, :], in0=z_big[0:TAIL, 8, :],
                        scalar1=box["inv"][0:TAIL, :],
                        scalar2=box["nb"][0:TAIL, :],
                        op0=ALU.mult, op1=ALU.add,
                    )
                    nc.sync.dma_start(
                        out=out_ext[s, 0, 8 * TO:H, :], in_=z_big[0:TAIL, 8, :]
                    )

                return [c1, c2, c3, c4,
                        norm_store(0, 4, "scalar"), norm_store(4, 8, "sync"),
                        c_tail]

            pending = []
            for s in range(B_PER_CORE):
                z_big = zp.tile([128, NT, W], F32, tag="z", name="z_big")
                stats = stat.tile([128, NT, 2, 6], F32, tag="stats", name="stats")
                for t in range(NT):
                    emit_tile(s, t, z_big, stats)
                    if pending:
                        pending.pop(0)()
                while pending:
                    pending.pop(0)()
                pending = finalize_chunks(s, z_big, stats)
            while pending:
                pending.pop(0)()
    nc.finalize()
    return nc


_NC_CACHE = {}


def _get_nc(mode, lo_passes):
    key = (mode,)
    if key not in _NC_CACHE:
        _NC_CACHE[key] = build_nc(mode, lo_passes)
    return _NC_CACHE[key]


def run(x, trace=False, mode="fp32", lo_passes=None, tmpdir=None):
    x = np.ascontiguousarray(np.asarray(x), dtype=np.float32)
    assert x.shape == (N_CORES * B_PER_CORE, 1, H, W), x.shape
    weights = _build_host_weights()
    in_maps = []
    for c in range(N_CORES):
        m = {"x": x[c * B_PER_CORE:(c + 1) * B_PER_CORE]}
        m.update(weights)
        in_maps.append(m)
    nc = _get_nc(mode, lo_passes)
    res = run_bass_kernel_spmd(
        nc, in_maps, list(range(N_CORES)), trace=trace, tmpdir=tmpdir
    )
    out = np.concatenate([res.results[c]["out"] for c in range(N_CORES)], axis=0)
    return out, res


def kernel(x):
    out, _ = run(x, trace=False)
    return out



# revision 11
# speedup vs baseline: 1.4441x; 1.4441x over previous
"""Trainium2 Bass kernel for the LTPE block:

    out_j = conv3x3(x, kernel_j)   (8 kernels: [-1 at neighbor j, +1 at center])
    out   = sum_j ((out_j + 1) * 0.5) * (2**j / 255)
    out   = InstanceNorm2d(out)    (per-sample over H,W, eps=1e-5, no affine)

Math: sum_j 2**j/255 == 1, so
    out = 0.5*(x - conv) + 0.5,  conv = sum_j (2**j/255) * shift_j(x)
InstanceNorm is invariant to the affine: with z = 255*x - sum_j 2**j*shift_j(x)
    result = (z - mean(z)) / sqrt(var(z) + 260100e-5)
z is computed as a 3x3 stencil via banded [128,128] fp32 matmuls (one per
column shift; walrus lowers fp32 matmuls to HI/LO passes on the PE, keeping
near-fp32 accuracy with no on-chip operand splitting).  Pure data parallel:
4 samples per NeuronCore, 8 cores.

Row tiling: tile t computes output rows [126t, 126t+126) (last tile: 16 rows)
from input rows [126t-1, 126t+127).  Output row 126t+n sits at partition n;
the vertical taps form a banded matrix with band (0,1,2) for t>0 and
(-1,0,1) for t=0 (zero-pad rows handled by band clipping / K=17 on the tail).

Samples are software-pipelined at tile granularity: the finalize chain of
sample s-1 (stats aggregation, normalize, store) is emitted in small chunks
between the tile emissions of sample s.  Input loads and output stores are
split across both HWDGE queues (sync + scalar engines).
"""

import numpy as np

import concourse.bass as bass
import concourse.tile as tile
from concourse import mybir
from concourse.bacc import Bacc
from concourse.bass_utils import run_bass_kernel_spmd

N_CORES = 8
B_PER_CORE = 4
H = W = 1024
TO = 126           # output rows per tile (input rows = TO + 2 halo)
NT = 9             # 8 full tiles + 16-row tail
TAIL = H - 8 * TO  # 16
EPS_P = 260100e-5  # 255^2 * 4 * 1e-5 : the InstanceNorm eps after rescaling

# neighbor offsets (dy, dx) for weights 2**j
_OFFSETS = [(0, -1), (1, -1), (1, 0), (1, 1), (0, 1), (-1, 1), (-1, 0), (-1, -1)]

F32 = mybir.dt.float32
F32R = mybir.dt.float32r
ALU = mybir.AluOpType
AF = mybir.ActivationFunctionType


def _build_host_weights():
    """Banded matrices V[dx][k, n]: coefficient of input partition k for
    output partition n, for column shift dx.  Band "a" (t=0): input row at
    partition k is row k, out row n -> taps k=n+dy.  Band "b" (t>0): input
    row at partition k is 126t-1+k, out row 126t+n -> taps k=n+1+dy."""
    out = {}
    for name, shift in (("a", 0), ("b", 1)):
        V = {dx: np.zeros((128, 128), np.float32) for dx in (-1, 0, 1)}
        for n in range(128):
            k = n + shift
            if k < 128:
                V[0][k, n] = 255.0  # center tap (+255 x)
        for j, (dy, dx) in enumerate(_OFFSETS):
            for n in range(128):
                k = n + shift + dy
                if 0 <= k < 128:
                    V[dx][k, n] += -float(2 ** j)
        for dx, tag in ((-1, "l"), (0, "c"), (1, "r")):
            out[f"v{tag}{name}"] = np.ascontiguousarray(V[dx], dtype=np.float32)

    # cross-partition count weights: row k weighted n_k / (H*W); all 128
    # output columns identical -> the matmul broadcasts the totals.
    counts = np.zeros((128,), np.float64)
    for t in range(NT):
        n_out = TO if t < 8 else TAIL
        counts[0:n_out] += W
    wcnt = np.tile((counts / float(H * W)).astype(np.float32)[:, None], (1, 128))
    out["wcnt"] = np.ascontiguousarray(wcnt, dtype=np.float32)
    return out


def _mm_cols(vname, h):
    """(in_c0, in_c1, out_c0, out_c1) for weight vname on PSUM half h:
    column shifts realized by sliding the moving operand's columns."""
    c0 = 512 * h
    if vname == "vc":
        return (c0, c0 + 512, 0, 512)
    if vname == "vl":
        return (0, 511, 1, 512) if h == 0 else (511, 1023, 0, 512)
    return (1, 513, 0, 512) if h == 0 else (513, 1024, 0, 511)


def build_nc(mode="fp32", lo_passes=None):
    nc = Bacc()
    # "bf16": stencil weights are exact in bf16 (+-2**j, 255); x is cast
    # f32->bf16 during the SWDGE DMA load, so the PE runs 1-pass bf16
    # matmuls instead of the 2x2-pass fp32 HI/LO lowering.  (fp32r would
    # need even-aligned even-sized column windows, which the +-1 column
    # shifts can't satisfy: 's3d3_mm_fp32r_restrictions'.)
    BF16 = mybir.dt.bfloat16
    MMDT = BF16 if mode == "bf16" else F32
    x_in = nc.declare_dram_parameter("x", [B_PER_CORE, 1, H, W], F32, isOutput=False)
    out_ext = nc.declare_dram_parameter("out", [B_PER_CORE, 1, H, W], F32, isOutput=True)
    w_names = ["vla", "vca", "vra", "vlb", "vcb", "vrb"]
    w_dram = {
        n: nc.declare_dram_parameter(n, [128, 128], MMDT, isOutput=False)
        for n in w_names
    }
    wcnt_d = nc.declare_dram_parameter("wcnt", [128, 128], F32, isOutput=False)

    def in_rows(t):
        in_a = max(TO * t - 1, 0)
        in_b = min(TO * t + TO + 1, H)
        return in_a, in_b

    def in_dma_engine(t):
        if mode == "bf16":
            return nc.gpsimd  # SWDGE: casts f32->bf16 in the DMA datapath
        return nc.sync if t % 2 == 0 else nc.scalar

    with tile.TileContext(nc) as tc:
        with (
            tc.tile_pool(name="singles", bufs=1) as singles,
            tc.tile_pool(name="xp", bufs=10) as xp,
            tc.tile_pool(name="zp", bufs=2) as zp,
            tc.tile_pool(name="stat", bufs=2) as stat,
            tc.tile_pool(name="sm", bufs=4) as sm,
            tc.tile_pool(name="psp", bufs=3, space="PSUM") as psp,
            tc.tile_pool(name="pss", bufs=1, space="PSUM") as pss,
        ):
            # prefetch the first tiles of sample 0 ahead of the weight loads
            prefetched = {}
            for pf_t in (0, 1, 2, 3):
                in_a, in_b = in_rows(pf_t)
                xt = xp.tile([128, W], MMDT, tag="xt", name=f"xt_pf{pf_t}")
                in_dma_engine(pf_t).dma_start(
                    out=xt[0:in_b - in_a, :], in_=x_in[0, 0, in_a:in_b, :]
                )
                prefetched[(0, pf_t)] = xt

            sb_v = {}
            for n in w_names:
                t_ = singles.tile([128, 128], MMDT, tag=n)
                nc.sync.dma_start(out=t_, in_=w_dram[n][:, :])
                sb_v[n] = t_
            sb_wcnt = singles.tile([128, 128], F32, tag="wcnt")
            nc.sync.dma_start(out=sb_wcnt, in_=wcnt_d[:, :])
            sb_eps = singles.tile([128, 1], F32, tag="eps")
            nc.vector.memset(sb_eps, EPS_P)

            def emit_tile(s, t, z_big, stats):
                if t == 0:
                    nc.gpsimd.memset(stats[:], 0.0)
                n_out = TO if t < 8 else TAIL
                in_a, in_b = in_rows(t)
                rows = in_b - in_a              # 127/128, or 17 on the tail
                K = rows                        # contraction depth
                band = "a" if t == 0 else "b"

                if (s, t) in prefetched:
                    xt = prefetched.pop((s, t))
                else:
                    xt = xp.tile([128, W], MMDT, tag="xt")
                    in_dma_engine(t).dma_start(
                        out=xt[0:rows, :], in_=x_in[s, 0, in_a:in_b, :]
                    )

                ps = psp.tile([128, 2, 512], F32, tag="ps")
                plan = [(v, h) for v in ("vc", "vl", "vr") for h in (0, 1)]
                for i, (vname, h) in enumerate(plan):
                    a, b, oa, ob = _mm_cols(vname, h)
                    nc.tensor.matmul(
                        ps[:, h, oa:ob],
                        lhsT=sb_v[vname + band][0:K, :],
                        rhs=xt[0:K, a:b],
                        start=(i < 2),
                        stop=(i >= len(plan) - 2),
                        skip_group_check=True,
                    )

                nc.scalar.copy(
                    out=z_big[0:n_out, t, :].rearrange("p (g f) -> p g f", f=512),
                    in_=ps[0:n_out, :, :],
                )
                for g in (0, 1):
                    nc.vector.bn_stats(
                        out=stats[0:n_out, t, g, :],
                        in_=ps[0:n_out, g, :],
                    )

            def finalize_chunks(s, z_big, stats):
                box = {}

                def c1():
                    mv = box["mv"] = sm.tile([128, 2], F32, tag="mv", name="mv")
                    nc.vector.memset(mv, 0.0)
                    nc.vector.bn_aggr(out=mv[0:TO, :], in_=stats[0:TO, :, :, :])
                    msq = sm.tile([128, 1], F32, tag="msq")
                    nc.vector.tensor_mul(msq, mv[:, 0:1], mv[:, 0:1])
                    nc.vector.tensor_add(mv[:, 1:2], mv[:, 1:2], msq)  # E2

                def c2():
                    tot_ps = pss.tile([128, 2], F32, tag="totps")
                    nc.tensor.matmul(
                        tot_ps[:, :], lhsT=sb_wcnt[:, :], rhs=box["mv"][:, :],
                        start=True, stop=True,
                    )
                    tot = box["tot"] = sm.tile([128, 2], F32, tag="tot", name="tot")
                    nc.scalar.copy(out=tot, in_=tot_ps)

                def c3():
                    tot = box["tot"]
                    m2 = sm.tile([128, 1], F32, tag="m2")
                    nc.vector.tensor_mul(m2, tot[:, 0:1], tot[:, 0:1])
                    var = sm.tile([128, 1], F32, tag="var")
                    nc.vector.tensor_sub(var, tot[:, 1:2], m2)
                    sd = box["sd"] = sm.tile([128, 1], F32, tag="sd", name="sd")
                    nc.scalar.activation(
                        out=sd, in_=var, func=AF.Sqrt, bias=sb_eps, scale=1.0
                    )

                def c4():
                    inv = box["inv"] = sm.tile([128, 1], F32, tag="inv", name="inv")
                    nc.vector.reciprocal(inv, box["sd"])
                    nbias = box["nb"] = sm.tile([128, 1], F32, tag="nb", name="nb")
                    nc.vector.tensor_scalar(
                        out=nbias, in0=inv, scalar1=box["tot"][:, 0:1],
                        scalar2=-1.0, op0=ALU.mult, op1=ALU.mult,
                    )

                def norm_store(t0, t1, eng_name):
                    def c():
                        nc.vector.tensor_scalar(
                            out=z_big[0:TO, t0:t1, :], in0=z_big[0:TO, t0:t1, :],
                            scalar1=box["inv"][0:TO, :],
                            scalar2=box["nb"][0:TO, :],
                            op0=ALU.mult, op1=ALU.add,
                        )
                        # output row 126t+n <-> (n, t) of z_big
                        eng = nc.sync if eng_name == "sync" else nc.scalar
                        eng.dma_start(
                            out=out_ext[s, 0, TO * t0:TO * t1, :].rearrange(
                                "(t n) w -> n t w", n=TO
                            ),
                            in_=z_big[0:TO, t0:t1, :],
                        )
                    return c

                def c_tail():
                    nc.vector.tensor_scalar(
                        out=z_big[0:TAIL, 8, :], in0=z_big[0:TAIL, 8, :],
                        scalar1=box["inv"][0:TAIL, :],
                        scalar2=box["nb"][0:TAIL, :],
                        op0=ALU.mult, op1=ALU.add,
                    )
                    nc.sync.dma_start(
                        out=out_ext[s, 0, 8 * TO:H, :], in_=z_big[0:TAIL, 8, :]
                    )

                return [c1, c2, c3, c4,
                        norm_store(0, 4, "scalar"), norm_store(4, 8, "sync"),
                        c_tail]

            pending = []
            for s in range(B_PER_CORE):
                z_big = zp.tile([128, NT, W], F32, tag="z", name="z_big")
                stats = stat.tile([128, NT, 2, 6], F32, tag="stats", name="stats")
                for t in range(NT):
                    emit_tile(s, t, z_big, stats)
                    if pending:
                        pending.pop(0)()
                while pending:
                    pending.pop(0)()
                pending = finalize_chunks(s, z_big, stats)
            while pending:
                pending.pop(0)()
    nc.finalize()
    return nc


_NC_CACHE = {}


def _get_nc(mode, lo_passes):
    key = (mode,)
    if key not in _NC_CACHE:
        _NC_CACHE[key] = build_nc(mode, lo_passes)
    return _NC_CACHE[key]


def run(x, trace=False, mode="fp32", lo_passes=None, tmpdir=None):
    x = np.ascontiguousarray(np.asarray(x), dtype=np.float32)
    assert x.shape == (N_CORES * B_PER_CORE, 1, H, W), x.shape
    weights = _build_host_weights()
    if mode == "bf16":
        import ml_dtypes

        for n in ("vla", "vca", "vra", "vlb", "vcb", "vrb"):
            weights[n] = np.ascontiguousarray(
                weights[n].astype(ml_dtypes.bfloat16)
            )
    in_maps = []
    for c in range(N_CORES):
        m = {"x": x[c * B_PER_CORE:(c + 1) * B_PER_CORE]}
        m.update(weights)
        in_maps.append(m)
    nc = _get_nc(mode, lo_passes)
    res = run_bass_kernel_spmd(
        nc, in_maps, list(range(N_CORES)), trace=trace, tmpdir=tmpdir
    )
    out = np.concatenate([res.results[c]["out"] for c in range(N_CORES)], axis=0)
    return out, res


def kernel(x):
    out, _ = run(x, trace=False)
    return out

